# revision 15
# baseline (speedup 1.0000x reference)
"""AttentionDecoder (4-layer GPT block stack) on 8 trn2 NeuronCores.

Sharding: data-parallel over batch (4) x 2-way Megatron tensor-parallel.
Core pair (2b, 2b+1) owns batch element b; within a pair, core t takes
heads 8t..8t+7 (QKV column split), the matching w_proj rows, w_fc column
half and w_fcp row half, and w_f column half.

v3 design:
- 512-wide token chunks (2 per layer); N=512 matmuls everywhere.
- Softmax denominators folded into the AV matmul via a ones-augmented V
  (per-head 65-column blocks, ones last -> denominator at pav row 64).
- LN/softmax row broadcasts via gpsimd.partition_broadcast.
- Causal slivers: score/exp/AV restricted to the unmasked query range of
  each 128-key block; one shared [128,128] triangular mask tile.
- Residual xT is ONE [128, 8*1024] tile; each AllReduce result (two
  512KB feature-half pieces) folds in with a single strided
  accumulate-DMA per piece.
- Attention (ACT-bound) is interleaved instruction-by-instruction with
  dense matmul chains (QKV of the next chunk / MLP of the previous one)
  so the PE stays busy and HAM stays at full clock.
"""

import sys

for _p in ("/opt/trn_rl_repo", "/root/.axon_site/_ro/trn_rl_repo"):
    if _p not in sys.path:
        sys.path.append(_p)

from contextlib import ExitStack

import numpy as np
import ml_dtypes

import concourse.bass as bass
import concourse.mybir as mybir
import concourse.tile as tile
from concourse import bacc
from concourse import bass_utils

F32 = mybir.dt.float32
BF16 = mybir.dt.bfloat16
NPBF16 = ml_dtypes.bfloat16

B, S, D, L = 4, 1024, 1024, 4
H, HD = 16, 64          # global heads, head dim
NH = H // 2             # heads per core (8)
P = 128                 # partitions
KT = D // P             # 8 k-tiles over the model dim
CH = 512                # token-chunk width (matmul moving dim)
NCH = S // CH           # 2 chunks
EPS = 1e-5
NEG = -30000.0
REPLICA_GROUPS = [[0, 1], [2, 3], [4, 5], [6, 7]]

AF = mybir.ActivationFunctionType
ADD = mybir.AluOpType.add
MULT = mybir.AluOpType.mult


def _build_nc():
    nc = bacc.Bacc("TRN2", target_bir_lowering=False, debug=False,
                   num_devices=8)

    def inp(name, shape, dt=BF16):
        return nc.declare_dram_parameter(name, list(shape), dt, isOutput=False)

    xT_d = inp("xT", [D, S], BF16)
    wqk_d = [inp(f"wqk_{i}", [D, 2 * NH * HD]) for i in range(L)]
    wv_d = [inp(f"wv_{i}", [D, NH * HD]) for i in range(L)]
    wproj_d = [inp(f"wproj_{i}", [NH * HD, D]) for i in range(L)]
    wfc_d = [inp(f"wfc_{i}", [D, 2048]) for i in range(L)]
    wfcp_d = [inp(f"wfcp_{i}", [2048, D]) for i in range(L)]
    wf_d = inp("wf", [D, 512])
    bqk_d = [inp(f"bqk_{i}", [P, 8], F32) for i in range(L)]
    bv_d = [inp(f"bv_{i}", [1, 512], BF16) for i in range(L)]
    bproj_d = [inp(f"bproj_{i}", [P, 8], F32) for i in range(L)]
    bfc_d = [inp(f"bfc_{i}", [P, 16], F32) for i in range(L)]
    bfcp_d = [inp(f"bfcp_{i}", [P, 8], F32) for i in range(L)]
    bf_d = inp("bf", [P, 4], F32)
    mask_d = inp("masks", [P, P], BF16)   # tri: NEG below diagonal (f < p)

    out_d = nc.declare_dram_parameter("out", [512, S], F32, isOutput=True)

    with tile.TileContext(nc) as tc, ExitStack() as ctx:
        resid = ctx.enter_context(tc.tile_pool(name="resid", bufs=1))
        wpool = ctx.enter_context(tc.tile_pool(name="wpool", bufs=1))
        spool = ctx.enter_context(tc.tile_pool(name="spool", bufs=1))
        hpool = ctx.enter_context(tc.tile_pool(name="hpool", bufs=8))
        lpool = ctx.enter_context(tc.tile_pool(name="lpool", bufs=1))
        sqpool = ctx.enter_context(tc.tile_pool(name="sqpool", bufs=2))
        qpool = ctx.enter_context(tc.tile_pool(name="qpool", bufs=2))
        kvpool = ctx.enter_context(tc.tile_pool(name="kvpool", bufs=1))
        epool = ctx.enter_context(tc.tile_pool(name="epool", bufs=3))
        apool = ctx.enter_context(tc.tile_pool(name="apool", bufs=2))
        mpool = ctx.enter_context(tc.tile_pool(name="mpool", bufs=16))
        arpool = ctx.enter_context(tc.tile_pool(name="arpool", bufs=3))
        rowpool = ctx.enter_context(tc.tile_pool(name="rowpool", bufs=1))
        bbpool = ctx.enter_context(tc.tile_pool(name="bbpool", bufs=2))
        opool = ctx.enter_context(tc.tile_pool(name="opool", bufs=1))
        ps_big = ctx.enter_context(tc.tile_pool(name="ps_big", bufs=4, space="PSUM"))
        ps_av = ctx.enter_context(tc.tile_pool(name="ps_av", bufs=2, space="PSUM"))
        ps_st = ctx.enter_context(tc.tile_pool(name="ps_st", bufs=1, space="PSUM"))
        dpool = ctx.enter_context(tc.tile_pool(name="dpool", bufs=2, space="DRAM"))

        # ---- constants ----
        ones_b = spool.tile([P, 1], BF16, tag="ones_b")
        nc.vector.memset(ones_b, 1.0)
        eps_t = spool.tile([1, 1], F32, tag="eps")
        nc.vector.memset(eps_t, EPS)
        mask_sb = spool.tile([P, P], BF16, tag="mask", name="mask")
        nc.sync.dma_start(out=mask_sb, in_=mask_d.ap())

        # warm up the collective firmware while the prologue computes
        warm_sb = spool.tile([P, 4], BF16, tag="warm")
        nc.vector.memset(warm_sb, 0.0)
        warm_in = dpool.tile([P, 4], BF16, tag="warm_in", name="warm_in")
        warm_out = dpool.tile([P, 4], BF16, tag="warm_out", name="warm_out")
        nc.sync.dma_start(out=warm_in, in_=warm_sb)
        nc.gpsimd.collective_compute(
            "AllReduce", ADD, ins=[warm_in.opt()], outs=[warm_out.opt()],
            replica_groups=REPLICA_GROUPS)

        # ---- resident residual stream: ONE tile, feature-major ----
        # column space: d*S + token  (d = feature-tile 0..7)
        xT_all = resid.tile([P, KT * S], BF16, tag="xT", name="xT_all")

        def xs(d, qc):
            """xT slice for feature-tile d, token chunk qc."""
            return xT_all[:, bass.ds(d * S + qc * CH, CH)]

        for d in range(KT):
            nc.sync.dma_start(out=xT_all[:, bass.ds(d * S, S)],
                              in_=xT_d[d * P:(d + 1) * P, :])

        # =========== helpers ===========

        def accum(qc, src_dram):
            """Fold a completed AR result (two [4P, CH] dram half pieces)
            into xT chunk qc: HWDGE load into a stage tile + DVE adds
            (keeps the gpsimd queue free for broadcasts)."""
            for half in range(2):
                stage = spool.tile([P, 4 * CH], BF16, tag="fold", name="fold")
                in_view = src_dram[half][:, :].rearrange(
                    "(d p) c -> p d c", p=P)
                nc.sync.dma_start(
                    out=stage[:, :].rearrange("p (d c) -> p d c", c=CH),
                    in_=in_view)
                for r in range(4):
                    d = half * 4 + r
                    nc.vector.tensor_add(xs(d, qc), xs(d, qc),
                                         stage[:, bass.ds(r * CH, CH)])

        def stats_chains(qc):
            """PE chains for LN stats (squares must already be emitted)."""
            ssum = ps_st.tile([1, CH], F32, tag="stsum", name="ssum")
            for d in range(KT):
                nc.tensor.matmul(ssum, ones_b, xs(d, qc),
                                 start=(d == 0), stop=(d == KT - 1))
            return ssum

        def stats_sq(qc, sqs):
            ssq = ps_st.tile([1, CH], F32, tag="stsq", name="ssq")
            for d in range(KT):
                nc.tensor.matmul(ssq, ones_b, sqs[d],
                                 start=(d == 0), stop=(d == KT - 1))
            return ssq

        def squares(qc, on_act=False):
            sqs = []
            for d in range(KT):
                sq = sqpool.tile([P, CH], BF16, tag="sq", name="sq")
                if on_act:
                    nc.scalar.activation(out=sq, in_=xs(d, qc), func=AF.Square)
                else:
                    nc.vector.tensor_mul(sq, xs(d, qc), xs(d, qc))
                sqs.append(sq)
            return sqs

        def ln_bcast(ssum, ssq):
            """Row math -> [P, 2CH] bf16 broadcast tile (rstd | -mean*rstd)."""
            mean = rowpool.tile([1, CH], F32, tag="rowA", name="mean")
            msq = rowpool.tile([1, CH], F32, tag="rowB", name="msq")
            var = rowpool.tile([1, CH], F32, tag="rowC", name="var")
            rstd = rowpool.tile([1, CH], F32, tag="rowD", name="rstd")
            nmr = rowpool.tile([1, CH], F32, tag="rowE", name="nmr")
            nc.vector.tensor_scalar_mul(mean, ssum, 1.0 / D)
            nc.vector.tensor_scalar_mul(msq, ssq, 1.0 / D)
            nc.vector.tensor_mul(var, mean, mean)
            nc.vector.tensor_sub(var, msq, var)
            nc.scalar.activation(out=msq, in_=var, func=AF.Sqrt, bias=eps_t)
            nc.vector.reciprocal_approx_fast(rstd, msq)
            nc.vector.scalar_tensor_tensor(out=nmr, in0=mean, scalar=-1.0,
                                           in1=rstd, op0=MULT, op1=MULT)
            pair = rowpool.tile([1, 2 * CH], BF16, tag="pair", name="pair")
            nc.vector.tensor_copy(pair[:, 0:CH], rstd)
            nc.vector.tensor_copy(pair[:, CH:2 * CH], nmr)
            bc = bbpool.tile([P, 2 * CH], BF16, tag="bcLN", name="bcLN")
            nc.gpsimd.partition_broadcast(bc[:, :], pair[:, :])
            return bc

        def ln_apply(qc, bc, tag, dlist):
            h_tiles = {}
            for d in dlist:
                t = lpool.tile([P, CH], BF16, tag="lnt", name="lnt")
                nc.vector.tensor_mul(t, xs(d, qc), bc[:, 0:CH])
                h = hpool.tile([P, CH], BF16, tag="h", name=f"h{tag}")
                nc.vector.tensor_add(h, t, bc[:, CH:2 * CH])
                h_tiles[d] = h
            return h_tiles

        def all_reduce_piece(tiles4, dtag):
            """AR four [P, CH] bf16 tiles as one 512KB piece."""
            ar_in = dpool.tile([4 * P, CH], BF16, tag=f"ari{dtag}",
                               name=f"ari{dtag}")
            ar_out = dpool.tile([4 * P, CH], BF16, tag=f"aro{dtag}",
                                name=f"aro{dtag}")
            for r in range(4):
                nc.sync.dma_start(out=ar_in[r * P:(r + 1) * P, :],
                                  in_=tiles4[r])
            nc.gpsimd.collective_compute(
                "AllReduce", ADD, ins=[ar_in.opt()], outs=[ar_out.opt()],
                replica_groups=REPLICA_GROUPS)
            return ar_out

        def all_reduce_halves(tiles, dtag):
            """AR the 8 [P, CH] bf16 tiles as two 512KB feature-half pieces."""
            return [all_reduce_piece(tiles[h * 4:(h + 1) * 4], f"{dtag}{h}")
                    for h in range(2)]

        # =================== layers ===================
        pending_mlp = [None] * NCH   # per chunk: AR output halves to fold in

        for i in range(L):
            # ---- per-layer weights & biases ----
            bqk_sb = spool.tile([P, 8], F32, tag="bqk", name="bqk")
            bproj_sb = spool.tile([P, 8], F32, tag="bproj", name="bproj")
            bfc_sb = spool.tile([P, 16], F32, tag="bfc", name="bfc")
            bfcp_sb = spool.tile([P, 8], F32, tag="bfcp", name="bfcp")
            bv_row = spool.tile([1, 512], BF16, tag="bv_row", name="bv_row")
            for sb, dr in ((bqk_sb, bqk_d[i]), (bproj_sb, bproj_d[i]),
                           (bfc_sb, bfc_d[i]), (bfcp_sb, bfcp_d[i]),
                           (bv_row, bv_d[i])):
                nc.sync.dma_start(out=sb, in_=dr.ap())
            bvB = bbpool.tile([P, 512], BF16, tag="bvB", bufs=1, name="bvB")
            nc.gpsimd.partition_broadcast(bvB[:, :], bv_row[:, :])

            wqk_sb = [wpool.tile([P, 1024], BF16, tag=f"wqk{k}",
                                 name=f"wqk{k}_{i}") for k in range(KT)]
            wv_sb = [wpool.tile([P, 512], BF16, tag=f"wv{k}",
                                name=f"wv{k}_{i}") for k in range(KT)]
            wproj_sb = [wpool.tile([P, 1024], BF16, tag=f"wpj{k}",
                                   name=f"wpj{k}_{i}") for k in range(4)]
            wfc_sb = [wpool.tile([P, 2048], BF16, tag=f"wfc{k}",
                                 name=f"wfc{k}_{i}") for k in range(KT)]
            wfcp_sb = [wpool.tile([P, 1024], BF16, tag=f"wfp{k}",
                                  name=f"wfp{k}_{i}") for k in range(16)]
            for k in range(KT):
                nc.sync.dma_start(out=wqk_sb[k], in_=wqk_d[i][k * P:(k + 1) * P, :])
                nc.sync.dma_start(out=wv_sb[k], in_=wv_d[i][k * P:(k + 1) * P, :])
            for k in range(4):
                nc.sync.dma_start(out=wproj_sb[k], in_=wproj_d[i][k * P:(k + 1) * P, :])
            for k in range(KT):
                nc.sync.dma_start(out=wfc_sb[k], in_=wfc_d[i][k * P:(k + 1) * P, :])
            for k in range(16):
                nc.sync.dma_start(out=wfcp_sb[k], in_=wfcp_d[i][k * P:(k + 1) * P, :])

            kT_sb = [kvpool.tile([P, S], BF16, tag=f"kT{m}", name=f"kT{m}_{i}")
                     for m in range(4)]
            # v: per key-tile, head-major 65-col blocks; col 65h+64 == 1.0
            v_sb = [kvpool.tile([P, 8 * 65], BF16, tag=f"v{r}",
                                name=f"v{r}_{i}") for r in range(KT)]
            for r in range(KT):
                nc.vector.memset(v_sb[r], 1.0)

            def qkv_chain(qc, h1, m):
                """One QKV output m-tile chain (m<4: q, else k)."""
                cs = bass.ds(qc * CH, CH)
                pmm = ps_big.tile([P, CH], F32, tag="ps", name="pqkv")
                for k in range(KT):
                    nc.tensor.matmul(pmm, wqk_sb[k][:, m * P:(m + 1) * P],
                                     h1[k], start=(k == 0), stop=(k == KT - 1))
                if m < 4:
                    q = qpool.tile([P, CH], BF16, tag=f"qT{m}", name="qT")
                    nc.scalar.activation(out=q, in_=pmm, func=AF.Identity,
                                         bias=bqk_sb[:, m:m + 1])
                    return q
                nc.scalar.activation(out=kT_sb[m - 4][:, cs], in_=pmm,
                                     func=AF.Identity,
                                     bias=bqk_sb[:, m:m + 1])
                return None

            def v_chain(qc, h1, r):
                pmm = ps_big.tile([P, 512], F32, tag="ps", name="pv")
                for k in range(KT):
                    nc.tensor.matmul(pmm, h1[k][:, r * P:(r + 1) * P],
                                     wv_sb[k], start=(k == 0),
                                     stop=(k == KT - 1))
                vt = v_sb[qc * (CH // P) + r]
                vt_view = vt[:, :].rearrange("p (h x) -> p h x", x=65)[:, :, 0:64]
                nc.vector.tensor_add(
                    vt_view,
                    pmm[:, :].rearrange("p (h x) -> p h x", x=64),
                    bvB[:, :].rearrange("p (h x) -> p h x", x=64))

            def attention(qc, qT, fillers):
                """Emit attention for chunk qc; after each (hp,h) group pop
                one filler thunk (dense PE work) to keep the PE warm."""
                n_kt = (qc + 1) * (CH // P)
                attnT = []
                for hp in range(4):
                    at = apool.tile([P, CH], BF16, tag=f"at{hp}", name="attnT")
                    for h in range(2):
                        lh = 2 * hp + h
                        hs = bass.ds(h * HD, HD)
                        pav = ps_av.tile([65, CH], F32, tag="pav", name="pav")
                        es = []
                        for kt in range(n_kt):
                            rel = kt * P - qc * CH
                            q0 = max(rel, 0)
                            pss = ps_big.tile([P, CH], F32, tag="ps",
                                              name="pss")
                            nc.tensor.matmul(pss[:, q0:CH],
                                             kT_sb[hp][hs, kt * P:(kt + 1) * P],
                                             qT[hp][hs, q0:CH],
                                             start=True, stop=True,
                                             tile_position=(h * HD, 0))
                            if rel >= 0:
                                nc.vector.tensor_add(
                                    pss[:, q0:q0 + P], pss[:, q0:q0 + P],
                                    mask_sb)
                            e = epool.tile([P, CH], BF16, tag="e", name="e")
                            nc.scalar.activation(out=e[:, q0:CH],
                                                 in_=pss[:, q0:CH],
                                                 func=AF.Exp, scale=0.125)
                            es.append((e, q0))
                        for j, (e, q0) in enumerate(es):
                            nc.tensor.matmul(pav[:, q0:CH],
                                             v_sb[j][:, lh * 65:(lh + 1) * 65],
                                             e[:, q0:CH], start=(j == 0),
                                             stop=(j == n_kt - 1))
                        den0 = rowpool.tile([1, CH], F32, tag="den0",
                                            name="den0")
                        nc.scalar.copy(den0, pav[64:65, :])
                        rrow = rowpool.tile([1, CH], F32, tag="rrow",
                                            name="rrow")
                        nc.vector.reciprocal_approx_fast(rrow, den0)
                        rrow_b = rowpool.tile([1, CH], BF16, tag="rrowb",
                                              name="rrowb")
                        nc.scalar.copy(rrow_b, rrow)
                        rB = bbpool.tile([HD, CH], BF16, tag="rB", name="rB", bufs=1)
                        nc.gpsimd.partition_broadcast(rB[:, :], rrow_b[:, :])
                        nc.vector.tensor_mul(at[hs, :], pav[0:HD, :], rB)
                        if fillers:
                            fillers.pop(0)()
                    attnT.append(at)
                for f in fillers:
                    f()
                fillers.clear()
                return attnT

            def proj_chains(attnT, ms, out_list):
                for m in ms:
                    pmm = ps_big.tile([P, CH], F32, tag="ps", name="pprj")
                    for k in range(4):
                        nc.tensor.matmul(pmm, wproj_sb[k][:, m * P:(m + 1) * P],
                                         attnT[k], start=(k == 0), stop=(k == 3))
                    t = arpool.tile([P, CH], BF16, tag="ar", name="prj")
                    nc.scalar.activation(out=t, in_=pmm, func=AF.Identity,
                                         bias=bproj_sb[:, m:m + 1])
                    out_list.append(t)

            def proj_ar(attnT):
                prj = []
                proj_chains(attnT, range(KT), prj)
                return all_reduce_halves(prj, "p")

            def fc_chain(qc, h2, m):
                pmm = ps_big.tile([P, CH], F32, tag="ps", name="pfc")
                for k in range(KT):
                    nc.tensor.matmul(pmm, wfc_sb[k][:, m * P:(m + 1) * P],
                                     h2[k], start=(k == 0), stop=(k == KT - 1))
                t = mpool.tile([P, CH], BF16, tag="mt", name="mt")
                nc.scalar.activation(out=t, in_=pmm, func=AF.Relu,
                                     bias=bfc_sb[:, m:m + 1])
                return t

            def fcp_ar(mt):
                fcp = []
                for m in range(KT):
                    pmm = ps_big.tile([P, CH], F32, tag="ps", name="pfcp")
                    for k in range(16):
                        nc.tensor.matmul(pmm, wfcp_sb[k][:, m * P:(m + 1) * P],
                                         mt[k], start=(k == 0), stop=(k == 15))
                    t = arpool.tile([P, CH], BF16, tag="ar", name="fcp")
                    nc.scalar.activation(out=t, in_=pmm, func=AF.Identity,
                                         bias=bfcp_sb[:, m:m + 1])
                    fcp.append(t)
                return all_reduce_halves(fcp, "m")

            # ---- pipeline ----
            # [1] LN1(0) (+ fold in prev-layer MLP AR)
            if pending_mlp[0] is not None:
                accum(0, pending_mlp[0])
                pending_mlp[0] = None
            sq0 = squares(0)                     # layer start: DVE free
            st0s = stats_chains(0)
            st0q = stats_sq(0, sq0)
            bc0 = ln_bcast(st0s, st0q)
            h1_0 = ln_apply(0, bc0, "1", range(KT))
            h1_0 = [h1_0[d] for d in range(KT)]
            # [2] QKV(0) + V(0)
            qT0 = [qkv_chain(0, h1_0, m) for m in range(4)]
            for m in range(4, KT):
                qkv_chain(0, h1_0, m)
            for r in range(CH // P):
                v_chain(0, h1_0, r)

            # [3] attention(0), interleaved with LN1(1)+QKV(1)+V(1)
            state = {}

            def f_acc1():
                if pending_mlp[1] is not None:
                    accum(1, pending_mlp[1])
                    pending_mlp[1] = None
                state["sq1"] = squares(1, on_act=True)  # attn(0) window

            def f_st1():
                s = stats_chains(1)
                q = stats_sq(1, state["sq1"])
                state["bc1"] = ln_bcast(s, q)

            def f_ap1a():
                state["h1a"] = ln_apply(1, state["bc1"], "1", range(0, 4))

            def f_ap1b():
                state["h1b"] = ln_apply(1, state["bc1"], "1", range(4, KT))
                state["h1"] = [state["h1a"][d] for d in range(4)] + \
                    [state["h1b"][d] for d in range(4, KT)]

            def mk_qkv1(ms):
                def f():
                    for m in ms:
                        t = qkv_chain(1, state["h1"], m)
                        if m < 4:
                            state.setdefault("qT1", []).append(t)
                return f

            def mk_v1(rs):
                def f():
                    for r in rs:
                        v_chain(1, state["h1"], r)
                return f

            fill0 = [f_acc1, f_st1, f_ap1a, f_ap1b,
                     mk_qkv1([0, 1]), mk_qkv1([2, 3]), mk_qkv1([4, 5]),
                     mk_qkv1([6, 7])]
            at0 = attention(0, qT0, fill0)
            mk_v1([0, 1])()
            mk_v1([2, 3])()

            # [4] attention(1), interleaved with proj(0) front-loaded so
            # its AR pieces are in flight during the rest of attention(1)
            prj0 = []

            def mk_prj0_ar(ms, half):
                def f():
                    proj_chains(at0, ms, prj0)
                    state.setdefault("prj_ar0", []).append(
                        all_reduce_piece(prj0[half * 4:half * 4 + 4],
                                         f"p{half}"))
                return f

            fill1 = [mk_prj0_ar([0, 1, 2, 3], 0), mk_prj0_ar([4, 5, 6, 7], 1)]
            at1 = attention(1, state["qT1"], fill1)
            # [5] proj(1) + AR
            prj_ar1 = proj_ar(at1)
            # [6] LN2(0) + MLP(0) + AR
            accum(0, state["prj_ar0"])
            sq20 = squares(0, on_act=True)
            st20s = stats_chains(0)
            st20q = stats_sq(0, sq20)
            bc20 = ln_bcast(st20s, st20q)
            h2_0 = ln_apply(0, bc20, "2", range(KT))
            h2_0 = [h2_0[d] for d in range(KT)]
            mt0 = [fc_chain(0, h2_0, m) for m in range(16)]
            pending_mlp[0] = fcp_ar(mt0)
            # [7] LN2(1) + MLP(1) + AR
            accum(1, prj_ar1)
            sq21 = squares(1)
            st21s = stats_chains(1)
            st21q = stats_sq(1, sq21)
            bc21 = ln_bcast(st21s, st21q)
            h2_1 = ln_apply(1, bc21, "2", range(KT))
            h2_1 = [h2_1[d] for d in range(KT)]
            mt1 = [fc_chain(1, h2_1, m) for m in range(16)]
            pending_mlp[1] = fcp_ar(mt1)

        # =================== final LN + head ===================
        bf_sb = spool.tile([P, 4], F32, tag="bf", name="bf_sb")
        nc.sync.dma_start(out=bf_sb, in_=bf_d.ap())
        wf_sb = [wpool.tile([P, 512], BF16, tag=f"wv{k}", name=f"wf{k}")
                 for k in range(KT)]
        for k in range(KT):
            nc.sync.dma_start(out=wf_sb[k], in_=wf_d[k * P:(k + 1) * P, :])
        for qc in range(NCH):
            cs = bass.ds(qc * CH, CH)
            accum(qc, pending_mlp[qc])
            pending_mlp[qc] = None
            sqf = squares(qc)
            stfs = stats_chains(qc)
            stfq = stats_sq(qc, sqf)
            bcf = ln_bcast(stfs, stfq)
            hf = ln_apply(qc, bcf, "f", range(KT))
            hf = [hf[d] for d in range(KT)]
            for m in range(4):
                pmm = ps_big.tile([P, CH], F32, tag="ps", name="phead")
                for k in range(KT):
                    nc.tensor.matmul(pmm, wf_sb[k][:, m * P:(m + 1) * P],
                                     hf[k], start=(k == 0), stop=(k == KT - 1))
                t = opool.tile([P, CH], F32, tag="oh", name="oh")
                nc.scalar.activation(out=t, in_=pmm, func=AF.Identity,
                                     bias=bf_sb[:, m:m + 1])
                nc.sync.dma_start(out=out_d[m * P:(m + 1) * P, cs], in_=t)

    nc.compile()
    return nc


_NC_CACHE = None


def _get_nc():
    global _NC_CACHE
    if _NC_CACHE is None:
        _NC_CACHE = _build_nc()
    return _NC_CACHE


def _rearr_vec(v):
    """[n*128] feature vector -> [128, n] (feature = m*128 + p)."""
    n = v.shape[0] // P
    return np.ascontiguousarray(v.reshape(n, P).T).astype(np.float32)


def _make_mask():
    p = np.arange(P)[:, None]
    f = np.arange(P)[None, :]
    return np.where(p <= f, 0.0, NEG).astype(NPBF16)


def _shard_inputs(x, ln1_g, ln1_b, w_attn, b_attn, w_proj, b_proj,
                  ln2_g, ln2_b, w_fc, b_fc, w_fcp, b_fcp,
                  lnf_g, lnf_b, w_f, b_f):
    bf = lambda a: np.ascontiguousarray(a).astype(NPBF16)
    mask = _make_mask()
    in_maps = []
    for core in range(8):
        b, t = core // 2, core % 2
        hsl = slice(t * NH * HD, (t + 1) * NH * HD)
        m = {"xT": np.ascontiguousarray(x[b].T).astype(NPBF16),
             "masks": mask,
             "wf": bf((w_f * lnf_g[:, None])[:, t * 512:(t + 1) * 512]),
             "bf": _rearr_vec((b_f + lnf_b @ w_f)[t * 512:(t + 1) * 512])}
        for i in range(L):
            g1, b1 = ln1_g[i], ln1_b[i]
            g2, b2 = ln2_g[i], ln2_b[i]
            wa = w_attn[i] * g1[:, None]                   # fold LN1 gamma
            ba = b_attn[i] + b1 @ w_attn[i]                # fold LN1 beta
            wq, wk, wv = wa[:, :D], wa[:, D:2 * D], wa[:, 2 * D:]
            bq, bk, bv = ba[:D], ba[D:2 * D], ba[2 * D:]
            m[f"wqk_{i}"] = bf(np.concatenate([wq[:, hsl], wk[:, hsl]], axis=1))
            m[f"wv_{i}"] = bf(wv[:, hsl])
            m[f"bqk_{i}"] = _rearr_vec(np.concatenate([bq[hsl], bk[hsl]]))
            m[f"bv_{i}"] = bv[hsl].reshape(1, 512).astype(NPBF16)
            m[f"wproj_{i}"] = bf(w_proj[i][hsl, :])
            m[f"bproj_{i}"] = _rearr_vec(b_proj[i] * 0.5)  # split across pair
            wfc_f = w_fc[i] * g2[:, None]
            bfc_f = b_fc[i] + b2 @ w_fc[i]
            m[f"wfc_{i}"] = bf(wfc_f[:, t * 2048:(t + 1) * 2048])
            m[f"bfc_{i}"] = _rearr_vec(bfc_f[t * 2048:(t + 1) * 2048])
            m[f"wfcp_{i}"] = bf(w_fcp[i][t * 2048:(t + 1) * 2048, :])
            m[f"bfcp_{i}"] = _rearr_vec(b_fcp[i] * 0.5)
        in_maps.append(m)
    return in_maps


def kernel(**inputs):
    nc = _get_nc()
    in_maps = _shard_inputs(**inputs)
    res = bass_utils.run_bass_kernel_spmd(nc, in_maps, core_ids=list(range(8)))
    outs = res.results
    full = np.empty((B, S, D), np.float32)
    for core in range(8):
        b, t = core // 2, core % 2
        full[b][:, t * 512:(t + 1) * 512] = outs[core]["out"].T
    return full


if __name__ == "__main__":
    nc = _get_nc()
    print("built ok;",
          sum(len(bb.instructions) for bb in nc.main_func.blocks
              if hasattr(bb, "instructions")), "instructions")
